# Initial kernel scaffold
#
"""GATv2 message-passing model (2 layers, fwd+bwd GAT + merge MLP + BN) on 8 TRN2 cores.

Strategy (edge-parallel, dst-sorted):
  - Edges of each direction are sorted by destination node and sharded across
    the 8 cores by contiguous dst ranges (1250 nodes/core, padded to 1344).
  - Per core, dst nodes are grouped into blocks of 112; each block's edges are
    packed into 128-edge tiles.  A combined matmul with lhsT=[edge_attr^T ;
    one_hot(dst)^T] (K=16+112=128) produces ee+xr per edge tile straight into
    PSUM; the gathered source features xl[src] (indirect DMA from a
    bf16 DRAM table) are injected with an identity matmul.
  - logits = att . leaky_relu(s) via ACT Lrelu + DVE mul/grouped-reduce.
    Softmax max-subtraction is dropped (mathematically exact; logits are O(1)).
  - Unnormalised aggregation: scatter-matmuls with lhsT = one_hot(dst)*exp(logit)
    accumulate into a per-block PSUM; normalisation by the segment sum happens
    per node block (divide by denom, mean over heads).
  - Node-level work (xl/xr/merge-MLP/BN) is node-sharded; xl is AllGathered
    (bf16) per layer/direction; BN stats use a tiny AllReduce.
"""

import os
import sys
from contextlib import ExitStack

import numpy as np
import ml_dtypes

for _p in ("/opt/trn_rl_repo",):
    if _p not in sys.path and os.path.isdir(_p):
        sys.path.append(_p)

import concourse.bass as bass
import concourse.bacc as bacc
import concourse.tile as tile
from concourse import mybir
from concourse.masks import make_identity
from concourse import bass_utils

BF16 = ml_dtypes.bfloat16
F32 = np.float32
DT = mybir.dt

NEG_SLOPE = 0.2
EPS = 1e-5


# ----------------------------------------------------------------------------
# Config
# ----------------------------------------------------------------------------
class Cfg:
    def __init__(self, N=10000, E=100000, DIN=128, H=4, C=128, ED=16, L=2, NC=8):
        assert DIN == 128 and H * C == 512 and ED == 16
        self.N, self.E, self.DIN, self.H, self.C, self.ED, self.L, self.NC = (
            N, E, DIN, H, C, ED, L, NC)
        self.HC = H * C
        self.NPC = (N + NC - 1) // NC          # real nodes per core
        self.BLK = 112                          # dst-nodes per scatter block
        self.NB = (self.NPC + self.BLK - 1) // self.BLK   # blocks per core
        self.NPAD = self.NB * self.BLK          # padded nodes per core
        self.NTAB = NC * self.NPAD              # rows in gathered xl tables


# ----------------------------------------------------------------------------
# Host-side preprocessing
# ----------------------------------------------------------------------------
def _prep_direction(cfg, edges, edge_attr):
    """Sort/shard/block/tile the edges of one direction.

    Returns (tiles_per_block [NB], per-core dict arrays).
    """
    NC, NPC, BLK, NB = cfg.NC, cfg.NPC, cfg.BLK, cfg.NB
    src, dst = np.asarray(edges[0]), np.asarray(edges[1])
    order = np.argsort(dst, kind="stable")
    s_src, s_dst = src[order], dst[order]
    core = np.minimum(s_dst // NPC, NC - 1)

    # edge counts per (core, block)
    counts = np.zeros((NC, NB), dtype=np.int64)
    per_core_edges = []
    for k in range(NC):
        sel = core == k
        ls, ld = s_src[sel], s_dst[sel] - k * NPC
        ea = edge_attr[order[sel]]
        blk = ld // BLK
        per_core_edges.append((ls, ld, ea, blk))
        cb = np.bincount(blk, minlength=NB)
        counts[k, : len(cb)] = cb[:NB]

    tiles_per_block = np.maximum(1, -(-counts.max(axis=0) // 128))  # ceil
    TT = int(tiles_per_block.sum())
    t_off = np.concatenate([[0], np.cumsum(tiles_per_block)]).astype(np.int64)

    out = []
    for k in range(NC):
        ls, ld, ea, blk = per_core_edges[k]
        gidx = np.zeros((128, TT), dtype=np.int32)
        comb = np.zeros((TT, 128, 128), dtype=BF16)
        scat = np.zeros((TT, 128, BLK), dtype=BF16)
        for b in range(NB):
            sel = blk == b
            nsel = int(sel.sum())
            if nsel == 0:
                continue
            j = np.arange(nsel)
            t = t_off[b] + j // 128
            p = j % 128
            srcs = ls[sel]
            # row index in the padded xl table
            rows = (srcs // NPC) * cfg.NPAD + (srcs % NPC)
            gidx[p, t] = rows.astype(np.int32)
            comb[t, 112:128, p] = ea[sel].astype(BF16)
            loc = (ld[sel] - b * BLK).astype(np.int64)
            comb[t, loc, p] = BF16(1.0)
            scat[t, p, loc] = BF16(1.0)
        out.append(dict(gidx=gidx, comb=comb, scat=scat))
    return [int(x) for x in tiles_per_block], out


def preprocess(cfg, inputs):
    """Build per-core in_maps + meta (tile counts)."""
    NC, NPC, NPAD, L = cfg.NC, cfg.NPC, cfg.NPAD, cfg.L
    x = np.asarray(inputs["x"], dtype=F32)

    meta = {}
    per_core = [dict() for _ in range(NC)]

    # node features, transposed + padded, per core
    for k in range(NC):
        xs = x[k * NPC: min((k + 1) * NPC, cfg.N)]
        xt = np.zeros((cfg.DIN, NPAD), dtype=BF16)
        xt[:, : xs.shape[0]] = xs.T.astype(BF16)
        per_core[k]["hT0"] = xt

    for d, ekey in (("f", "fwd_edges_index"), ("b", "bwd_edges_index")):
        tpb, arrs = _prep_direction(cfg, np.asarray(inputs[ekey]),
                                    np.asarray(inputs["edge_attr"], dtype=F32))
        meta[f"tpb_{d}"] = tpb
        for k in range(NC):
            per_core[k][f"gidx_{d}"] = arrs[k]["gidx"]
            per_core[k][f"comb_{d}"] = arrs[k]["comb"]
            per_core[k][f"scat_{d}"] = arrs[k]["scat"]

        # weights for this direction (replicated on all cores)
        Wl = np.asarray(inputs[f"Wl_{d}"], dtype=F32)
        Wr = np.asarray(inputs[f"Wr_{d}"], dtype=F32)
        We = np.asarray(inputs[f"We_{d}"], dtype=F32)
        att = np.asarray(inputs[f"att_{d}"], dtype=F32)     # [L,H,C]
        gb = np.asarray(inputs[f"bias_{d}"], dtype=F32)     # [L,C]
        bl = np.asarray(inputs[f"bl_{d}"], dtype=F32)
        br = np.asarray(inputs[f"br_{d}"], dtype=F32)
        meta[f"has_sbias_{d}"] = bool(np.any(bl) or np.any(br))
        attb = np.broadcast_to(att.reshape(L, 1, cfg.HC), (L, 128, cfg.HC))
        sb = (bl + br).reshape(L, 1, cfg.HC)
        sbias = np.broadcast_to(sb, (L, 128, cfg.HC))
        # msg path misses the +bl of xl = h@Wl + bl; after softmax-normalise
        # and head-mean that is exactly +mean_h(bl) per channel -> fold into
        # the GAT output bias.
        gb = gb + bl.reshape(L, cfg.H, cfg.C).mean(axis=1)
        for k in range(NC):
            per_core[k][f"Wl_{d}"] = Wl.astype(BF16)
            per_core[k][f"Wr_{d}"] = Wr.astype(BF16)
            per_core[k][f"We_{d}"] = We.astype(BF16)
            per_core[k][f"attb_{d}"] = np.ascontiguousarray(attb.astype(BF16))
            per_core[k][f"gatb_{d}"] = np.ascontiguousarray(gb.reshape(L, cfg.C, 1))
            if meta[f"has_sbias_{d}"]:
                per_core[k][f"sbias_{d}"] = np.ascontiguousarray(sbias.astype(F32))

    Wm1 = np.asarray(inputs["Wm1"], dtype=F32)   # [L, 2C, C]
    Wm2 = np.asarray(inputs["Wm2"], dtype=F32)   # [L, C, C]
    bm1 = np.asarray(inputs["bm1"], dtype=F32)   # [L, C]
    gamma = np.asarray(inputs["gamma"], dtype=F32)
    beta = np.asarray(inputs["beta"], dtype=F32)
    # bm2 is dropped: BN is shift-invariant.
    for k in range(NC):
        per_core[k]["Wm1f"] = Wm1[:, : cfg.C].astype(BF16)
        per_core[k]["Wm1b"] = Wm1[:, cfg.C:].astype(BF16)
        per_core[k]["Wm2"] = Wm2.astype(BF16)
        per_core[k]["bm1"] = np.ascontiguousarray(bm1.reshape(L, cfg.C, 1))
        per_core[k]["gamma"] = np.ascontiguousarray(gamma.reshape(L, cfg.C, 1))
        per_core[k]["beta"] = np.ascontiguousarray(beta.reshape(L, cfg.C, 1))
    return per_core, meta


# ----------------------------------------------------------------------------
# Program builder
# ----------------------------------------------------------------------------
def build_program(cfg, meta, in_shapes):
    """Emit the full 2-layer program.  Same program for all 8 cores (SPMD)."""
    NC, NB, BLK, NPAD, NPC, HC, L, H, C = (cfg.NC, cfg.NB, cfg.BLK, cfg.NPAD,
                                           cfg.NPC, cfg.HC, cfg.L, cfg.H, cfg.C)
    nc = bacc.Bacc("TRN2", target_bir_lowering=False, debug=False,
                   num_devices=NC)
    rg = [list(range(NC))]

    DT_MAP = {np.dtype(np.float32): DT.float32, np.dtype(BF16): DT.bfloat16,
              np.dtype(np.int32): DT.int32}
    inp = {}
    for name, (shape, dt) in in_shapes.items():
        inp[name] = nc.dram_tensor(name, list(shape), DT_MAP[np.dtype(dt)],
                                   kind="ExternalInput").ap()
    out_dram = nc.dram_tensor("out", [NPC, cfg.DIN], DT.float32,
                              kind="ExternalOutput").ap()

    # internal DRAM
    xtab = {}
    xloc = {}
    for l in range(L):
        for d in "fb":
            xtab[l, d] = nc.dram_tensor(f"xtab{l}{d}", [cfg.NTAB, HC],
                                        DT.bfloat16, kind="Internal",
                                        addr_space="Shared").ap()
            xloc[l, d] = nc.dram_tensor(f"xloc{l}{d}", [NPAD, HC],
                                        DT.bfloat16, kind="Internal").ap()
    bn_in = [nc.dram_tensor(f"bnin{l}", [cfg.C, 2], DT.float32,
                            kind="Internal").ap() for l in range(L)]
    bn_out = [nc.dram_tensor(f"bnout{l}", [cfg.C, 2], DT.float32,
                             kind="Internal", addr_space="Shared").ap()
              for l in range(L)]

    with tile.TileContext(nc) as tc, ExitStack() as ctx:
        sb = ctx.enter_context(tc.tile_pool(name="sb", bufs=1))
        sb2 = ctx.enter_context(tc.tile_pool(name="sb2", bufs=2))
        sb3 = ctx.enter_context(tc.tile_pool(name="sb3", bufs=3))
        ps = ctx.enter_context(tc.tile_pool(name="ps", bufs=2, space="PSUM"))
        pse = ctx.enter_context(tc.tile_pool(name="pse", bufs=2, space="PSUM"))

        # constants
        alpha_sb = sb.tile([128, 1], DT.float32, name="alpha")
        nc.vector.memset(alpha_sb[:], NEG_SLOPE)
        id_bf = sb.tile([128, 128], DT.bfloat16, name="id_bf")
        id_f32 = sb.tile([128, 128], DT.float32, name="id_f32")
        make_identity(nc, id_bf[:])
        make_identity(nc, id_f32[:])

        # persistent SBUF state across a layer
        hT = sb.tile([128, NPAD], DT.bfloat16, name="hT", bufs=2)
        nc.sync.dma_start(hT[:], inp["hT0"][:, :])

        gidx_sb = {}
        for d in "fb":
            TT = sum(meta[f"tpb_{d}"])
            gidx_sb[d] = sb.tile([128, TT], DT.int32, name=f"gidx{d}")
            nc.sync.dma_start(gidx_sb[d][:], inp[f"gidx_{d}"][:, :])

        fT_all = {d: sb.tile([128, NPAD], DT.bfloat16, name=f"fT_{d}", bufs=2)
                  for d in "fb"}

        def node_phase(l, d, hT_cur):
            """xl -> xloc (DRAM) + AG; xr -> comb_rhs rows 16:128."""
            Wl_sb = sb2.tile([128, HC], DT.bfloat16, tag="wl")
            Wr_sb = sb2.tile([128, HC], DT.bfloat16, tag="wr")
            nc.sync.dma_start(Wl_sb[:], inp[f"Wl_{d}"][l])
            nc.sync.dma_start(Wr_sb[:], inp[f"Wr_{d}"][l])
            comb_rhs = sb2.tile([128, NB * HC], DT.bfloat16, tag=f"crhs{d}")
            for b in range(NB):
                nc.sync.dma_start(comb_rhs[112:128, b * HC:(b + 1) * HC],
                                  inp[f"We_{d}"][l])
            xl_all = sb2.tile([BLK, NB * HC], DT.bfloat16, tag="xlall")
            for b in range(NB):
                sl = slice(b * BLK, (b + 1) * BLK)
                ps_n = ps.tile([BLK, HC], DT.float32, tag="psn", space="PSUM")
                nc.tensor.matmul(out=ps_n[:], lhsT=hT_cur[:, sl], rhs=Wl_sb[:],
                                 start=True, stop=True)
                nc.scalar.activation(xl_all[:, b * HC:(b + 1) * HC], ps_n[:],
                                     mybir.ActivationFunctionType.Identity)

                ps_n2 = ps.tile([BLK, HC], DT.float32, tag="psn", space="PSUM")
                nc.tensor.matmul(out=ps_n2[:], lhsT=hT_cur[:, sl], rhs=Wr_sb[:],
                                 start=True, stop=True)
                nc.scalar.activation(
                    comb_rhs[0:BLK, b * HC:(b + 1) * HC], ps_n2[:],
                    mybir.ActivationFunctionType.Identity)
            nc.sync.dma_start(
                xloc[l, d][:, :].rearrange("(b p) c -> p b c", b=NB),
                xl_all[:].rearrange("p (b c) -> p b c", b=NB))
            # AllGather local xl into the shared table
            nc.gpsimd.collective_compute(
                "AllGather", mybir.AluOpType.bypass, replica_groups=rg,
                ins=[xloc[l, d][:, :]], outs=[xtab[l, d][:, :]])
            return comb_rhs

        def edge_phase(l, d, comb_rhs):
            """Per-block gather + attention + aggregation -> fT_all[d]."""
            tpb = meta[f"tpb_{d}"]
            attb_sb = sb2.tile([128, HC], DT.bfloat16, tag="attb")
            nc.sync.dma_start(attb_sb[:], inp[f"attb_{d}"][l])
            gatb_sb = sb2.tile([C, 1], DT.float32, tag="gatb")
            nc.sync.dma_start(gatb_sb[:], inp[f"gatb_{d}"][l])
            sbias_sb = None
            if meta[f"has_sbias_{d}"]:
                sbias_sb = sb2.tile([128, HC], DT.float32, tag="sbias")
                nc.sync.dma_start(sbias_sb[:], inp[f"sbias_{d}"][l])
            t0 = 0
            for b in range(NB):
                Tb = tpb[b]
                gx = sb2.tile([128, Tb * HC], DT.bfloat16, tag="gx")
                for t in range(Tb):
                    nc.gpsimd.indirect_dma_start(
                        out=gx[:, t * HC:(t + 1) * HC], out_offset=None,
                        in_=xtab[l, d][:, :],
                        in_offset=bass.IndirectOffsetOnAxis(
                            ap=gidx_sb[d][:, t0 + t:t0 + t + 1], axis=0))
                compT = sb2.tile([128, Tb * 128], DT.bfloat16, tag="compT")
                nc.sync.dma_start(
                    compT[:].rearrange("p (t q) -> p t q", t=Tb),
                    inp[f"comb_{d}"][t0:t0 + Tb].rearrange("t p q -> p t q"))
                scatT = sb2.tile([128, Tb * BLK], DT.bfloat16, tag="scatT")
                nc.sync.dma_start(
                    scatT[:].rearrange("p (t q) -> p t q", t=Tb),
                    inp[f"scat_{d}"][t0:t0 + Tb].rearrange("t p q -> p t q"))

                agg_ps = ps.tile([BLK, HC], DT.float32, tag="agg", space="PSUM")
                den_ps = ps.tile([BLK, H], DT.float32, tag="den", space="PSUM", bufs=1)
                for t in range(Tb):
                    ps_s = pse.tile([128, HC], DT.float32, tag="pss",
                                    space="PSUM")
                    nc.tensor.matmul(
                        out=ps_s[:], lhsT=compT[:, t * 128:(t + 1) * 128],
                        rhs=comb_rhs[:, b * HC:(b + 1) * HC],
                        start=True, stop=False)
                    nc.tensor.matmul(
                        out=ps_s[:], lhsT=id_bf[:],
                        rhs=gx[:, t * HC:(t + 1) * HC], start=False, stop=True)
                    z = sb3.tile([128, HC], DT.bfloat16, tag="z")
                    if sbias_sb is not None:
                        zf = sb3.tile([128, HC], DT.float32, tag="zf")
                        nc.vector.tensor_add(zf[:], ps_s[:], sbias_sb[:])
                        nc.scalar.activation(z[:], zf[:],
                                             mybir.ActivationFunctionType.Prelu,
                                             alpha=alpha_sb[:])
                    else:
                        nc.scalar.activation(z[:], ps_s[:],
                                             mybir.ActivationFunctionType.Prelu,
                                             alpha=alpha_sb[:])
                    zw = sb3.tile([128, HC], DT.bfloat16, tag="zw")
                    nc.vector.tensor_mul(zw[:], z[:], attb_sb[:])
                    logit = sb3.tile([128, H], DT.float32, tag="logit")
                    nc.vector.reduce_sum(
                        logit[:].rearrange("p (h o) -> p h o", o=1),
                        zw[:].rearrange("p (h c) -> p h c", h=H),
                        axis=mybir.AxisListType.X)
                    exl = sb3.tile([128, H], DT.bfloat16, tag="exl")
                    nc.scalar.activation(exl[:], logit[:],
                                         mybir.ActivationFunctionType.Exp)
                    exf = sb3.tile([128, H], DT.float32, tag="exf")
                    nc.scalar.activation(exf[:], logit[:],
                                         mybir.ActivationFunctionType.Exp)
                    scat_t = scatT[:, t * BLK:(t + 1) * BLK]
                    nc.tensor.matmul(out=den_ps[:], lhsT=scat_t, rhs=exl[:],
                                     start=(t == 0), stop=(t == Tb - 1))
                    for h in range(H):
                        sSh = sb3.tile([128, BLK], DT.bfloat16, tag=f"sS{h}")
                        nc.vector.tensor_scalar_mul(sSh[:], scat_t,
                                                    exf[:, h:h + 1])
                        nc.tensor.matmul(
                            out=agg_ps[:, h * C:(h + 1) * C], lhsT=sSh[:],
                            rhs=gx[:, t * HC + h * C: t * HC + (h + 1) * C],
                            start=(t == 0 and h == 0),
                            stop=(t == Tb - 1 and h == H - 1))
                # normalise + head mean
                den_sb = sb3.tile([BLK, H], DT.float32, tag="densb")
                nc.vector.tensor_scalar_add(den_sb[:], den_ps[:], 1e-30)
                dr = sb3.tile([BLK, H], DT.float32, tag="dr")
                nc.vector.reciprocal(dr[:], den_sb[:])
                aggN = sb3.tile([BLK, HC], DT.float32, tag="aggN")
                nc.vector.tensor_mul(
                    aggN[:].rearrange("p (h c) -> p h c", h=H),
                    agg_ps[:].rearrange("p (h c) -> p h c", h=H),
                    dr[:].rearrange("p (h o) -> p h o", o=1).to_broadcast([BLK, H, C]))
                fmean = sb3.tile([BLK, C], DT.float32, tag="fmean")
                nc.vector.reduce_sum(
                    fmean[:].rearrange("p (c o) -> p c o", o=1),
                    aggN[:].rearrange("p (h c) -> p c h", h=H),
                    axis=mybir.AxisListType.X)
                fT_ps = ps.tile([C, BLK], DT.float32, tag="ftps", space="PSUM", bufs=1)
                nc.tensor.transpose(fT_ps[:], fmean[:], id_f32[0:BLK, 0:BLK])
                nc.scalar.activation(
                    fT_all[d][:, b * BLK:(b + 1) * BLK], fT_ps[:],
                    mybir.ActivationFunctionType.Identity,
                    bias=gatb_sb[:], scale=1.0 / H)
                t0 += Tb

        def merge_bn(l, hT_next):
            Wm1f_sb = sb2.tile([C, C], DT.bfloat16, tag="wm1f")
            Wm1b_sb = sb2.tile([C, C], DT.bfloat16, tag="wm1b")
            Wm2_sb = sb2.tile([C, C], DT.bfloat16, tag="wm2")
            bm1_sb = sb2.tile([C, 1], DT.float32, tag="bm1")
            nc.sync.dma_start(Wm1f_sb[:], inp["Wm1f"][l])
            nc.sync.dma_start(Wm1b_sb[:], inp["Wm1b"][l])
            nc.sync.dma_start(Wm2_sb[:], inp["Wm2"][l])
            nc.sync.dma_start(bm1_sb[:], inp["bm1"][l])
            y_all = sb2.tile([C, NPAD], DT.float32, tag="yall")
            for b in range(NB):
                sl = slice(b * BLK, (b + 1) * BLK)
                ps_m = ps.tile([C, BLK], DT.float32, tag="psn", space="PSUM")
                nc.tensor.matmul(out=ps_m[:], lhsT=Wm1f_sb[:],
                                 rhs=fT_all["f"][:, sl], start=True, stop=False)
                nc.tensor.matmul(out=ps_m[:], lhsT=Wm1b_sb[:],
                                 rhs=fT_all["b"][:, sl], start=False, stop=True)
                mT = sb3.tile([C, BLK], DT.bfloat16, tag="mT")
                nc.scalar.activation(mT[:], ps_m[:],
                                     mybir.ActivationFunctionType.Relu,
                                     bias=bm1_sb[:])
                ps_y = ps.tile([C, BLK], DT.float32, tag="psn", space="PSUM")
                nc.tensor.matmul(out=ps_y[:], lhsT=Wm2_sb[:], rhs=mT[:],
                                 start=True, stop=True)
                nc.scalar.activation(y_all[:, sl], ps_y[:],
                                     mybir.ActivationFunctionType.Identity)
            # BN stats over the real nodes
            stats = sb3.tile([C, 2], DT.float32, tag="stats")
            nc.vector.reduce_sum(stats[:, 0:1], y_all[:, 0:NPC],
                                 axis=mybir.AxisListType.X)
            sqscr = sb3.tile([C, NPC], DT.float32, tag="sqscr")
            nc.scalar.activation(sqscr[:], y_all[:, 0:NPC],
                                 mybir.ActivationFunctionType.Square,
                                 accum_out=stats[:, 1:2])
            nc.sync.dma_start(bn_in[l][:, :], stats[:])
            nc.gpsimd.collective_compute(
                "AllReduce", mybir.AluOpType.add, replica_groups=rg,
                ins=[bn_in[l][:, :]], outs=[bn_out[l][:, :]])
            stg = sb3.tile([C, 2], DT.float32, tag="stg")
            nc.sync.dma_start(stg[:], bn_out[l][:, :])
            gam = sb3.tile([C, 1], DT.float32, tag="gam")
            bet = sb3.tile([C, 1], DT.float32, tag="bet")
            nc.sync.dma_start(gam[:], inp["gamma"][l])
            nc.sync.dma_start(bet[:], inp["beta"][l])
            mu = sb3.tile([C, 1], DT.float32, tag="mu")
            nc.vector.tensor_scalar_mul(mu[:], stg[:, 0:1], 1.0 / cfg.N)
            ex2 = sb3.tile([C, 1], DT.float32, tag="ex2")
            nc.vector.tensor_scalar_mul(ex2[:], stg[:, 1:2], 1.0 / cfg.N)
            mu2 = sb3.tile([C, 1], DT.float32, tag="mu2")
            nc.vector.tensor_mul(mu2[:], mu[:], mu[:])
            var = sb3.tile([C, 1], DT.float32, tag="var")
            nc.vector.tensor_tensor(var[:], ex2[:], mu2[:],
                                    op=mybir.AluOpType.subtract)
            vare = sb3.tile([C, 1], DT.float32, tag="vare")
            nc.vector.tensor_scalar_add(vare[:], var[:], EPS)
            sd = sb3.tile([C, 1], DT.float32, tag="sd")
            nc.scalar.activation(sd[:], vare[:],
                                 mybir.ActivationFunctionType.Sqrt)
            rstd = sb3.tile([C, 1], DT.float32, tag="rstd")
            nc.vector.reciprocal(rstd[:], sd[:])
            scale = sb3.tile([C, 1], DT.float32, tag="scale")
            nc.vector.tensor_mul(scale[:], rstd[:], gam[:])
            nmu = sb3.tile([C, 1], DT.float32, tag="nmu")
            nc.vector.tensor_mul(nmu[:], mu[:], scale[:])
            bias = sb3.tile([C, 1], DT.float32, tag="bias")
            nc.vector.tensor_tensor(bias[:], bet[:], nmu[:],
                                    op=mybir.AluOpType.subtract)
            nc.scalar.activation(hT_next[:], y_all[:],
                                 mybir.ActivationFunctionType.Relu,
                                 bias=bias[:], scale=scale[:])

        # ------------------- main flow -------------------
        hT_cur = hT
        for l in range(L):
            comb_rhs = {}
            for d in "fb":
                comb_rhs[d] = node_phase(l, d, hT_cur)
            for d in "fb":
                edge_phase(l, d, comb_rhs[d])
            hdt = DT.bfloat16 if l < L - 1 else DT.float32
            hT_next = sb.tile([128, NPAD], hdt, name="hT", bufs=2)
            merge_bn(l, hT_next)
            hT_cur = hT_next

        # final transpose + output
        out_all = sb.tile([BLK, NB * 128], DT.float32, name="out_all")
        for b in range(NB):
            tp = ps.tile([BLK, 128], DT.float32, tag="ftps", space="PSUM", bufs=1)
            nc.tensor.transpose(tp[:], hT_cur[:, b * BLK:(b + 1) * BLK], id_f32[:])
            nc.scalar.activation(out_all[:, b * 128:(b + 1) * 128], tp[:],
                                 mybir.ActivationFunctionType.Identity)
        nfull = NPC // BLK  # full blocks
        nc.sync.dma_start(
            out_dram[0:nfull * BLK].rearrange("(b p) c -> p b c", b=nfull),
            out_all[:, 0:nfull * 128].rearrange("p (b c) -> p b c", b=nfull))
        tail = NPC - nfull * BLK
        if tail > 0:
            nc.sync.dma_start(out_dram[nfull * BLK:NPC],
                              out_all[0:tail, nfull * 128:(nfull + 1) * 128])

    nc.compile()
    return nc


# ----------------------------------------------------------------------------
# Entry point
# ----------------------------------------------------------------------------
_CACHE = {}


def _run(cfg, inputs):
    per_core, meta = preprocess(cfg, inputs)
    in_shapes = {k: (v.shape, v.dtype) for k, v in per_core[0].items()}
    key = (cfg.N, cfg.E, tuple(meta["tpb_f"]), tuple(meta["tpb_b"]),
           meta["has_sbias_f"], meta["has_sbias_b"])
    if key not in _CACHE:
        _CACHE[key] = build_program(cfg, meta, in_shapes)
    nc = _CACHE[key]
    res = bass_utils.run_bass_kernel_spmd(nc, per_core,
                                          core_ids=list(range(cfg.NC)))
    outs = [res.results[k]["out"][: cfg.NPC] for k in range(cfg.NC)]
    full = np.concatenate(outs, axis=0)[: cfg.N].astype(np.float32)
    return full, nc, per_core, meta


def kernel(**inputs) -> np.ndarray:
    cfg = Cfg(N=int(inputs["x"].shape[0]), E=int(inputs["edge_attr"].shape[0]))
    out, _, _, _ = _run(cfg, inputs)
    return out



# revision 1
# speedup vs baseline: 1.2977x; 1.2977x over previous
"""GATv2 message-passing model (2 layers, fwd+bwd GAT + merge MLP + BN) on 8 TRN2 cores.

Strategy (edge-parallel, dst-sorted):
  - Edges of each direction are sorted by destination node and sharded across
    the 8 cores by contiguous dst ranges (1250 nodes/core, padded to 1344).
  - Per core, dst nodes are grouped into blocks of 112; each block's edges are
    packed into 128-edge tiles.  A combined matmul with lhsT=[edge_attr^T ;
    one_hot(dst)^T] (K=16+112=128) produces ee+xr per edge tile straight into
    PSUM; the gathered source features xl[src] (indirect DMA from a
    bf16 DRAM table) are injected with an identity matmul.
  - logits = att . leaky_relu(s) via ACT Lrelu + DVE mul/grouped-reduce.
    Softmax max-subtraction is dropped (mathematically exact; logits are O(1)).
  - Unnormalised aggregation: scatter-matmuls with lhsT = one_hot(dst)*exp(logit)
    accumulate into a per-block PSUM; normalisation by the segment sum happens
    per node block (divide by denom, mean over heads).
  - Node-level work (xl/xr/merge-MLP/BN) is node-sharded; xl is AllGathered
    (bf16) per layer/direction; BN stats use a tiny AllReduce.
"""

import os
import sys
from contextlib import ExitStack

import numpy as np
import ml_dtypes

for _p in ("/opt/trn_rl_repo",):
    if _p not in sys.path and os.path.isdir(_p):
        sys.path.append(_p)

import concourse.bass as bass
import concourse.bacc as bacc
import concourse.tile as tile
from concourse import mybir
from concourse.masks import make_identity
from concourse import bass_utils

BF16 = ml_dtypes.bfloat16
F32 = np.float32
DT = mybir.dt

NEG_SLOPE = 0.2
EPS = 1e-5


# ----------------------------------------------------------------------------
# Config
# ----------------------------------------------------------------------------
class Cfg:
    def __init__(self, N=10000, E=100000, DIN=128, H=4, C=128, ED=16, L=2, NC=8):
        assert DIN == 128 and H * C == 512 and ED == 16
        self.N, self.E, self.DIN, self.H, self.C, self.ED, self.L, self.NC = (
            N, E, DIN, H, C, ED, L, NC)
        self.HC = H * C
        self.NPC = (N + NC - 1) // NC          # real nodes per core
        self.BLK = 112                          # dst-nodes per scatter block
        self.NB = (self.NPC + self.BLK - 1) // self.BLK   # blocks per core
        self.NPAD = self.NB * self.BLK          # padded nodes per core
        self.NTAB = NC * self.NPAD              # rows in gathered xl tables


# ----------------------------------------------------------------------------
# Host-side preprocessing
# ----------------------------------------------------------------------------
def _prep_direction(cfg, edges, edge_attr):
    """Sort/shard/block/tile the edges of one direction.

    Returns (tiles_per_block [NB], per-core dict arrays).
    """
    NC, NPC, BLK, NB = cfg.NC, cfg.NPC, cfg.BLK, cfg.NB
    src, dst = np.asarray(edges[0]), np.asarray(edges[1])
    order = np.argsort(dst, kind="stable")
    s_src, s_dst = src[order], dst[order]
    core = np.minimum(s_dst // NPC, NC - 1)

    # edge counts per (core, block)
    counts = np.zeros((NC, NB), dtype=np.int64)
    per_core_edges = []
    for k in range(NC):
        sel = core == k
        ls, ld = s_src[sel], s_dst[sel] - k * NPC
        ea = edge_attr[order[sel]]
        blk = ld // BLK
        per_core_edges.append((ls, ld, ea, blk))
        cb = np.bincount(blk, minlength=NB)
        counts[k, : len(cb)] = cb[:NB]

    tiles_per_block = np.maximum(1, -(-counts.max(axis=0) // 128))  # ceil
    TT = int(tiles_per_block.sum())
    t_off = np.concatenate([[0], np.cumsum(tiles_per_block)]).astype(np.int64)

    out = []
    for k in range(NC):
        ls, ld, ea, blk = per_core_edges[k]
        gidx = np.zeros((128, TT), dtype=np.int32)
        comb = np.zeros((TT, 128, 128), dtype=BF16)
        scat = np.zeros((TT, 128, BLK), dtype=BF16)
        for b in range(NB):
            sel = blk == b
            nsel = int(sel.sum())
            if nsel == 0:
                continue
            j = np.arange(nsel)
            t = t_off[b] + j // 128
            p = j % 128
            srcs = ls[sel]
            # row index in the padded xl table
            rows = (srcs // NPC) * cfg.NPAD + (srcs % NPC)
            gidx[p, t] = rows.astype(np.int32)
            comb[t, 112:128, p] = ea[sel].astype(BF16)
            loc = (ld[sel] - b * BLK).astype(np.int64)
            comb[t, loc, p] = BF16(1.0)
            scat[t, p, loc] = BF16(1.0)
        out.append(dict(gidx=gidx, comb=comb, scat=scat))
    return [int(x) for x in tiles_per_block], out


def preprocess(cfg, inputs):
    """Build per-core in_maps + meta (tile counts)."""
    NC, NPC, NPAD, L = cfg.NC, cfg.NPC, cfg.NPAD, cfg.L
    x = np.asarray(inputs["x"], dtype=F32)

    meta = {}
    per_core = [dict() for _ in range(NC)]

    # node features, transposed + padded, per core
    for k in range(NC):
        xs = x[k * NPC: min((k + 1) * NPC, cfg.N)]
        xt = np.zeros((cfg.DIN, NPAD), dtype=BF16)
        xt[:, : xs.shape[0]] = xs.T.astype(BF16)
        per_core[k]["hT0"] = xt

    for d, ekey in (("f", "fwd_edges_index"), ("b", "bwd_edges_index")):
        tpb, arrs = _prep_direction(cfg, np.asarray(inputs[ekey]),
                                    np.asarray(inputs["edge_attr"], dtype=F32))
        meta[f"tpb_{d}"] = tpb
        for k in range(NC):
            per_core[k][f"gidx_{d}"] = arrs[k]["gidx"]
            per_core[k][f"comb_{d}"] = arrs[k]["comb"]
            per_core[k][f"scat_{d}"] = arrs[k]["scat"]

        # weights for this direction (replicated on all cores)
        Wl = np.asarray(inputs[f"Wl_{d}"], dtype=F32)
        Wr = np.asarray(inputs[f"Wr_{d}"], dtype=F32)
        We = np.asarray(inputs[f"We_{d}"], dtype=F32)
        att = np.asarray(inputs[f"att_{d}"], dtype=F32)     # [L,H,C]
        gb = np.asarray(inputs[f"bias_{d}"], dtype=F32)     # [L,C]
        bl = np.asarray(inputs[f"bl_{d}"], dtype=F32)
        br = np.asarray(inputs[f"br_{d}"], dtype=F32)
        meta[f"has_sbias_{d}"] = bool(np.any(bl) or np.any(br))
        attb = np.broadcast_to(att.reshape(L, 1, cfg.HC), (L, 128, cfg.HC))
        sb = (bl + br).reshape(L, 1, cfg.HC)
        sbias = np.broadcast_to(sb, (L, 128, cfg.HC))
        # msg path misses the +bl of xl = h@Wl + bl; after softmax-normalise
        # and head-mean that is exactly +mean_h(bl) per channel -> fold into
        # the GAT output bias.
        gb = gb + bl.reshape(L, cfg.H, cfg.C).mean(axis=1)
        for k in range(NC):
            per_core[k][f"Wl_{d}"] = Wl.astype(BF16)
            per_core[k][f"Wr_{d}"] = Wr.astype(BF16)
            per_core[k][f"We_{d}"] = We.astype(BF16)
            per_core[k][f"attb_{d}"] = np.ascontiguousarray(attb.astype(BF16))
            per_core[k][f"gatb_{d}"] = np.ascontiguousarray(gb.reshape(L, cfg.C, 1))
            if meta[f"has_sbias_{d}"]:
                per_core[k][f"sbias_{d}"] = np.ascontiguousarray(sbias.astype(F32))

    Wm1 = np.asarray(inputs["Wm1"], dtype=F32)   # [L, 2C, C]
    Wm2 = np.asarray(inputs["Wm2"], dtype=F32)   # [L, C, C]
    bm1 = np.asarray(inputs["bm1"], dtype=F32)   # [L, C]
    gamma = np.asarray(inputs["gamma"], dtype=F32)
    beta = np.asarray(inputs["beta"], dtype=F32)
    # bm2 is dropped: BN is shift-invariant.
    for k in range(NC):
        per_core[k]["Wm1f"] = Wm1[:, : cfg.C].astype(BF16)
        per_core[k]["Wm1b"] = Wm1[:, cfg.C:].astype(BF16)
        per_core[k]["Wm2"] = Wm2.astype(BF16)
        per_core[k]["bm1"] = np.ascontiguousarray(bm1.reshape(L, cfg.C, 1))
        per_core[k]["gamma"] = np.ascontiguousarray(gamma.reshape(L, cfg.C, 1))
        per_core[k]["beta"] = np.ascontiguousarray(beta.reshape(L, cfg.C, 1))
    return per_core, meta


# ----------------------------------------------------------------------------
# Program builder
# ----------------------------------------------------------------------------
def build_program(cfg, meta, in_shapes):
    """Emit the full 2-layer program.  Same program for all 8 cores (SPMD)."""
    NC, NB, BLK, NPAD, NPC, HC, L, H, C = (cfg.NC, cfg.NB, cfg.BLK, cfg.NPAD,
                                           cfg.NPC, cfg.HC, cfg.L, cfg.H, cfg.C)
    nc = bacc.Bacc("TRN2", target_bir_lowering=False, debug=False,
                   num_devices=NC)
    rg = [list(range(NC))]

    DT_MAP = {np.dtype(np.float32): DT.float32, np.dtype(BF16): DT.bfloat16,
              np.dtype(np.int32): DT.int32}
    inp = {}
    for name, (shape, dt) in in_shapes.items():
        inp[name] = nc.dram_tensor(name, list(shape), DT_MAP[np.dtype(dt)],
                                   kind="ExternalInput").ap()
    out_dram = nc.dram_tensor("out", [NPC, cfg.DIN], DT.float32,
                              kind="ExternalOutput").ap()

    # internal DRAM
    xtab = {}
    xloc = {}
    for l in range(L):
        for d in "fb":
            xtab[l, d] = nc.dram_tensor(f"xtab{l}{d}", [cfg.NTAB, HC],
                                        DT.bfloat16, kind="Internal",
                                        addr_space="Shared").ap()
            xloc[l, d] = nc.dram_tensor(f"xloc{l}{d}", [NPAD, HC],
                                        DT.bfloat16, kind="Internal").ap()
    bn_in = [nc.dram_tensor(f"bnin{l}", [cfg.C, 2], DT.float32,
                            kind="Internal").ap() for l in range(L)]
    bn_out = [nc.dram_tensor(f"bnout{l}", [cfg.C, 2], DT.float32,
                             kind="Internal", addr_space="Shared").ap()
              for l in range(L)]

    with tile.TileContext(nc) as tc, ExitStack() as ctx:
        sb = ctx.enter_context(tc.tile_pool(name="sb", bufs=1))
        sb2 = ctx.enter_context(tc.tile_pool(name="sb2", bufs=2))
        sb3 = ctx.enter_context(tc.tile_pool(name="sb3", bufs=3))
        ps = ctx.enter_context(tc.tile_pool(name="ps", bufs=2, space="PSUM"))
        pse = ctx.enter_context(tc.tile_pool(name="pse", bufs=2, space="PSUM"))

        # constants
        alpha_sb = sb.tile([128, 1], DT.float32, name="alpha")
        nc.vector.memset(alpha_sb[:], NEG_SLOPE)
        id_bf = sb.tile([128, 128], DT.bfloat16, name="id_bf")
        id_f32 = sb.tile([128, 128], DT.float32, name="id_f32")
        make_identity(nc, id_bf[:])
        make_identity(nc, id_f32[:])

        # persistent SBUF state across a layer
        hT = sb.tile([128, NPAD], DT.bfloat16, name="hT", bufs=2)
        nc.sync.dma_start(hT[:], inp["hT0"][:, :])

        gidx_sb = {}
        for d in "fb":
            TT = sum(meta[f"tpb_{d}"])
            gidx_sb[d] = sb.tile([128, TT], DT.int32, name=f"gidx{d}")
            nc.sync.dma_start(gidx_sb[d][:], inp[f"gidx_{d}"][:, :])

        fT_all = {d: sb.tile([128, NPAD], DT.bfloat16, name=f"fT_{d}", bufs=2)
                  for d in "fb"}

        def node_phase(l, d, hT_cur):
            """xl -> xloc (DRAM) + AG; xr -> comb_rhs rows 16:128."""
            Wl_sb = sb2.tile([128, HC], DT.bfloat16, tag="wl")
            Wr_sb = sb2.tile([128, HC], DT.bfloat16, tag="wr")
            nc.sync.dma_start(Wl_sb[:], inp[f"Wl_{d}"][l])
            nc.sync.dma_start(Wr_sb[:], inp[f"Wr_{d}"][l])
            comb_rhs = sb2.tile([128, NB * HC], DT.bfloat16, tag=f"crhs{d}")
            for b in range(NB):
                nc.sync.dma_start(comb_rhs[112:128, b * HC:(b + 1) * HC],
                                  inp[f"We_{d}"][l])
            xl_all = sb2.tile([BLK, NB * HC], DT.bfloat16, tag="xlall")
            for b in range(NB):
                sl = slice(b * BLK, (b + 1) * BLK)
                ps_n = ps.tile([BLK, HC], DT.float32, tag="psn", space="PSUM")
                nc.tensor.matmul(out=ps_n[:], lhsT=hT_cur[:, sl], rhs=Wl_sb[:],
                                 start=True, stop=True)
                nc.scalar.activation(xl_all[:, b * HC:(b + 1) * HC], ps_n[:],
                                     mybir.ActivationFunctionType.Identity)

                ps_n2 = ps.tile([BLK, HC], DT.float32, tag="psn", space="PSUM")
                nc.tensor.matmul(out=ps_n2[:], lhsT=hT_cur[:, sl], rhs=Wr_sb[:],
                                 start=True, stop=True)
                nc.scalar.activation(
                    comb_rhs[0:BLK, b * HC:(b + 1) * HC], ps_n2[:],
                    mybir.ActivationFunctionType.Identity)
            nc.sync.dma_start(
                xloc[l, d][:, :].rearrange("(b p) c -> p b c", b=NB),
                xl_all[:].rearrange("p (b c) -> p b c", b=NB))
            # AllGather local xl into the shared table
            nc.gpsimd.collective_compute(
                "AllGather", mybir.AluOpType.bypass, replica_groups=rg,
                ins=[xloc[l, d][:, :]], outs=[xtab[l, d][:, :]])
            return comb_rhs

        def edge_phase(l, d, comb_rhs):
            """Per-block gather + attention + aggregation -> fT_all[d]."""
            tpb = meta[f"tpb_{d}"]
            attb_sb = sb2.tile([128, HC], DT.bfloat16, tag="attb")
            nc.sync.dma_start(attb_sb[:], inp[f"attb_{d}"][l])
            gatb_sb = sb2.tile([C, 1], DT.float32, tag="gatb")
            nc.sync.dma_start(gatb_sb[:], inp[f"gatb_{d}"][l])
            sbias_sb = None
            if meta[f"has_sbias_{d}"]:
                sbias_sb = sb2.tile([128, HC], DT.float32, tag="sbias")
                nc.sync.dma_start(sbias_sb[:], inp[f"sbias_{d}"][l])
            t0 = 0
            for b in range(NB):
                Tb = tpb[b]
                gx = sb2.tile([128, Tb * HC], DT.bfloat16, tag="gx")
                for t in range(Tb):
                    nc.gpsimd.indirect_dma_start(
                        out=gx[:, t * HC:(t + 1) * HC], out_offset=None,
                        in_=xtab[l, d][:, :],
                        in_offset=bass.IndirectOffsetOnAxis(
                            ap=gidx_sb[d][:, t0 + t:t0 + t + 1], axis=0))
                compT = sb2.tile([128, Tb * 128], DT.bfloat16, tag="compT")
                nc.sync.dma_start(
                    compT[:].rearrange("p (t q) -> p t q", t=Tb),
                    inp[f"comb_{d}"][t0:t0 + Tb].rearrange("t p q -> p t q"))
                scatT = sb2.tile([128, Tb * BLK], DT.bfloat16, tag="scatT")
                nc.sync.dma_start(
                    scatT[:].rearrange("p (t q) -> p t q", t=Tb),
                    inp[f"scat_{d}"][t0:t0 + Tb].rearrange("t p q -> p t q"))

                agg_ps = ps.tile([BLK, HC], DT.float32, tag="agg", space="PSUM")
                den_ps = ps.tile([BLK, H], DT.float32, tag="den", space="PSUM", bufs=1)
                for t in range(Tb):
                    ps_s = pse.tile([128, HC], DT.float32, tag="pss",
                                    space="PSUM")
                    nc.tensor.matmul(
                        out=ps_s[:], lhsT=compT[:, t * 128:(t + 1) * 128],
                        rhs=comb_rhs[:, b * HC:(b + 1) * HC],
                        start=True, stop=False)
                    nc.tensor.matmul(
                        out=ps_s[:], lhsT=id_bf[:],
                        rhs=gx[:, t * HC:(t + 1) * HC], start=False, stop=True)
                    z = sb3.tile([128, HC], DT.bfloat16, tag="z")
                    if sbias_sb is not None:
                        zf = sb3.tile([128, HC], DT.float32, tag="zf")
                        nc.vector.tensor_add(zf[:], ps_s[:], sbias_sb[:])
                        nc.scalar.activation(z[:], zf[:],
                                             mybir.ActivationFunctionType.Prelu,
                                             alpha=alpha_sb[:])
                    else:
                        nc.scalar.activation(z[:], ps_s[:],
                                             mybir.ActivationFunctionType.Prelu,
                                             alpha=alpha_sb[:])
                    zw = sb3.tile([128, HC], DT.bfloat16, tag="zw")
                    nc.vector.tensor_mul(zw[:], z[:], attb_sb[:])
                    logit = sb3.tile([128, H], DT.float32, tag="logit")
                    nc.vector.reduce_sum(
                        logit[:].rearrange("p (h o) -> p h o", o=1),
                        zw[:].rearrange("p (h c) -> p h c", h=H),
                        axis=mybir.AxisListType.X)
                    exl = sb3.tile([128, H], DT.bfloat16, tag="exl")
                    nc.scalar.activation(exl[:], logit[:],
                                         mybir.ActivationFunctionType.Exp)
                    exf = sb3.tile([128, H], DT.float32, tag="exf")
                    nc.scalar.activation(exf[:], logit[:],
                                         mybir.ActivationFunctionType.Exp)
                    scat_t = scatT[:, t * BLK:(t + 1) * BLK]
                    nc.tensor.matmul(out=den_ps[:], lhsT=scat_t, rhs=exl[:],
                                     start=(t == 0), stop=(t == Tb - 1))
                    for h in range(H):
                        sSh = sb3.tile([128, BLK], DT.bfloat16, tag=f"sS{h}")
                        nc.vector.tensor_scalar_mul(sSh[:], scat_t,
                                                    exf[:, h:h + 1])
                        nc.tensor.matmul(
                            out=agg_ps[:, h * C:(h + 1) * C], lhsT=sSh[:],
                            rhs=gx[:, t * HC + h * C: t * HC + (h + 1) * C],
                            start=(t == 0 and h == 0),
                            stop=(t == Tb - 1 and h == H - 1))
                # normalise + head mean
                den_sb = sb3.tile([BLK, H], DT.float32, tag="densb")
                nc.vector.tensor_scalar_add(den_sb[:], den_ps[:], 1e-30)
                dr = sb3.tile([BLK, H], DT.float32, tag="dr")
                nc.vector.reciprocal(dr[:], den_sb[:])
                aggN = sb3.tile([BLK, HC], DT.float32, tag="aggN")
                nc.vector.tensor_mul(
                    aggN[:].rearrange("p (h c) -> p h c", h=H),
                    agg_ps[:].rearrange("p (h c) -> p h c", h=H),
                    dr[:].rearrange("p (h o) -> p h o", o=1).to_broadcast([BLK, H, C]))
                fmean = sb3.tile([BLK, C], DT.float32, tag="fmean")
                nc.vector.reduce_sum(
                    fmean[:].rearrange("p (c o) -> p c o", o=1),
                    aggN[:].rearrange("p (h c) -> p c h", h=H),
                    axis=mybir.AxisListType.X)
                fT_ps = ps.tile([C, BLK], DT.float32, tag="ftps", space="PSUM", bufs=1)
                nc.tensor.transpose(fT_ps[:], fmean[:], id_f32[0:BLK, 0:BLK])
                nc.scalar.activation(
                    fT_all[d][:, b * BLK:(b + 1) * BLK], fT_ps[:],
                    mybir.ActivationFunctionType.Identity,
                    bias=gatb_sb[:], scale=1.0 / H)
                t0 += Tb

        def merge_bn(l, hT_next):
            Wm1f_sb = sb2.tile([C, C], DT.bfloat16, tag="wm1f")
            Wm1b_sb = sb2.tile([C, C], DT.bfloat16, tag="wm1b")
            Wm2_sb = sb2.tile([C, C], DT.bfloat16, tag="wm2")
            bm1_sb = sb2.tile([C, 1], DT.float32, tag="bm1")
            nc.sync.dma_start(Wm1f_sb[:], inp["Wm1f"][l])
            nc.sync.dma_start(Wm1b_sb[:], inp["Wm1b"][l])
            nc.sync.dma_start(Wm2_sb[:], inp["Wm2"][l])
            nc.sync.dma_start(bm1_sb[:], inp["bm1"][l])
            y_all = sb2.tile([C, NPAD], DT.float32, tag="yall")
            for b in range(NB):
                sl = slice(b * BLK, (b + 1) * BLK)
                ps_m = ps.tile([C, BLK], DT.float32, tag="psn", space="PSUM")
                nc.tensor.matmul(out=ps_m[:], lhsT=Wm1f_sb[:],
                                 rhs=fT_all["f"][:, sl], start=True, stop=False)
                nc.tensor.matmul(out=ps_m[:], lhsT=Wm1b_sb[:],
                                 rhs=fT_all["b"][:, sl], start=False, stop=True)
                mT = sb3.tile([C, BLK], DT.bfloat16, tag="mT")
                nc.scalar.activation(mT[:], ps_m[:],
                                     mybir.ActivationFunctionType.Relu,
                                     bias=bm1_sb[:])
                ps_y = ps.tile([C, BLK], DT.float32, tag="psn", space="PSUM")
                nc.tensor.matmul(out=ps_y[:], lhsT=Wm2_sb[:], rhs=mT[:],
                                 start=True, stop=True)
                nc.scalar.activation(y_all[:, sl], ps_y[:],
                                     mybir.ActivationFunctionType.Identity)
            # BN stats over the real nodes
            stats = sb3.tile([C, 2], DT.float32, tag="stats")
            nc.vector.reduce_sum(stats[:, 0:1], y_all[:, 0:NPC],
                                 axis=mybir.AxisListType.X)
            sqscr = sb3.tile([C, NPC], DT.float32, tag="sqscr")
            nc.scalar.activation(sqscr[:], y_all[:, 0:NPC],
                                 mybir.ActivationFunctionType.Square,
                                 accum_out=stats[:, 1:2])
            nc.sync.dma_start(bn_in[l][:, :], stats[:])
            nc.gpsimd.collective_compute(
                "AllReduce", mybir.AluOpType.add, replica_groups=rg,
                ins=[bn_in[l][:, :]], outs=[bn_out[l][:, :]])
            stg = sb3.tile([C, 2], DT.float32, tag="stg")
            nc.sync.dma_start(stg[:], bn_out[l][:, :])
            gam = sb3.tile([C, 1], DT.float32, tag="gam")
            bet = sb3.tile([C, 1], DT.float32, tag="bet")
            nc.sync.dma_start(gam[:], inp["gamma"][l])
            nc.sync.dma_start(bet[:], inp["beta"][l])
            mu = sb3.tile([C, 1], DT.float32, tag="mu")
            nc.vector.tensor_scalar_mul(mu[:], stg[:, 0:1], 1.0 / cfg.N)
            ex2 = sb3.tile([C, 1], DT.float32, tag="ex2")
            nc.vector.tensor_scalar_mul(ex2[:], stg[:, 1:2], 1.0 / cfg.N)
            mu2 = sb3.tile([C, 1], DT.float32, tag="mu2")
            nc.vector.tensor_mul(mu2[:], mu[:], mu[:])
            var = sb3.tile([C, 1], DT.float32, tag="var")
            nc.vector.tensor_tensor(var[:], ex2[:], mu2[:],
                                    op=mybir.AluOpType.subtract)
            vare = sb3.tile([C, 1], DT.float32, tag="vare")
            nc.vector.tensor_scalar_add(vare[:], var[:], EPS)
            sd = sb3.tile([C, 1], DT.float32, tag="sd")
            nc.scalar.activation(sd[:], vare[:],
                                 mybir.ActivationFunctionType.Sqrt)
            rstd = sb3.tile([C, 1], DT.float32, tag="rstd")
            nc.vector.reciprocal(rstd[:], sd[:])
            scale = sb3.tile([C, 1], DT.float32, tag="scale")
            nc.vector.tensor_mul(scale[:], rstd[:], gam[:])
            nmu = sb3.tile([C, 1], DT.float32, tag="nmu")
            nc.vector.tensor_mul(nmu[:], mu[:], scale[:])
            bias = sb3.tile([C, 1], DT.float32, tag="bias")
            nc.vector.tensor_tensor(bias[:], bet[:], nmu[:],
                                    op=mybir.AluOpType.subtract)
            nc.scalar.activation(hT_next[:], y_all[:],
                                 mybir.ActivationFunctionType.Relu,
                                 bias=bias[:], scale=scale[:])

        # ------------------- main flow -------------------
        hT_cur = hT
        for l in range(L):
            comb_rhs = {}
            for d in "fb":
                comb_rhs[d] = node_phase(l, d, hT_cur)
            for d in "fb":
                edge_phase(l, d, comb_rhs[d])
            hdt = DT.bfloat16 if l < L - 1 else DT.float32
            hT_next = sb.tile([128, NPAD], hdt, name="hT", bufs=2)
            merge_bn(l, hT_next)
            hT_cur = hT_next

        # final transpose + output
        out_all = sb.tile([BLK, NB * 128], DT.float32, name="out_all")
        for b in range(NB):
            tp = ps.tile([BLK, 128], DT.float32, tag="ftps", space="PSUM", bufs=1)
            nc.tensor.transpose(tp[:], hT_cur[:, b * BLK:(b + 1) * BLK], id_f32[:])
            nc.scalar.activation(out_all[:, b * 128:(b + 1) * 128], tp[:],
                                 mybir.ActivationFunctionType.Identity)
        nfull = NPC // BLK  # full blocks
        nc.sync.dma_start(
            out_dram[0:nfull * BLK].rearrange("(b p) c -> p b c", b=nfull),
            out_all[:, 0:nfull * 128].rearrange("p (b c) -> p b c", b=nfull))
        tail = NPC - nfull * BLK
        if tail > 0:
            nc.sync.dma_start(out_dram[nfull * BLK:NPC],
                              out_all[0:tail, nfull * 128:(nfull + 1) * 128])

    nc.compile()
    return nc


# ----------------------------------------------------------------------------
# Entry point
# ----------------------------------------------------------------------------
_CACHE = {}


def _run(cfg, inputs):
    per_core, meta = preprocess(cfg, inputs)
    in_shapes = {k: (v.shape, v.dtype) for k, v in per_core[0].items()}
    key = (cfg.N, cfg.E, tuple(meta["tpb_f"]), tuple(meta["tpb_b"]),
           meta["has_sbias_f"], meta["has_sbias_b"])
    if key not in _CACHE:
        _CACHE[key] = build_program(cfg, meta, in_shapes)
    nc = _CACHE[key]
    res = bass_utils.run_bass_kernel_spmd(nc, per_core,
                                          core_ids=list(range(cfg.NC)))
    outs = [res.results[k]["out"][: cfg.NPC] for k in range(cfg.NC)]
    full = np.concatenate(outs, axis=0)[: cfg.N].astype(np.float32)
    return full, nc, per_core, meta


def kernel(**inputs) -> np.ndarray:
    cfg = Cfg(N=int(inputs["x"].shape[0]), E=int(inputs["edge_attr"].shape[0]))
    out, _, _, _ = _run(cfg, inputs)
    return out

